# revision 9
# baseline (speedup 1.0000x reference)
import numpy as np
import jax
import jax.numpy as jnp
from functools import partial

# nn_GatedDeltaNet: B=1, T=4096, D=2048, HK=16, HV=32, DK=DV=128, K_CONV=4, CHUNK=64
# Sharding: heads tensor-parallel across 8 cores. Each core gets HK_L=2 key heads
# (HV_L=4 value heads): contiguous 1536-col slice of W_qkvz, 8-col slice of W_ba,
# matching conv_w rows, and a 512-row slice of W_out. Partial outputs are
# all-reduced across cores.

HK, HV, DK, DV, K_CONV, CHUNK = 16, 32, 128, 128, 4, 64
DIM = 2048
EPS = 1e-6
NC = 8
HK_L = HK // NC          # 2 key heads per core
HV_L = HV // NC          # 4 value heads per core
RATIO = HV // HK         # 2


def _l2norm(x):
    return x * jax.lax.rsqrt(jnp.sum(x * x, -1, keepdims=True) + EPS)


def _chunk_gated_delta_rule(q, k, v, g, beta):
    B, T, H, Dk = q.shape
    Dv = v.shape[-1]
    N = T // CHUNK
    scale = Dk ** -0.5

    def to_c(x):
        x = jnp.moveaxis(x, 1, 2)
        x = x.reshape(x.shape[0], x.shape[1], N, CHUNK, *x.shape[3:])
        return jnp.moveaxis(x, 2, 0)

    qc, kc, vc = to_c(q), to_c(k), to_c(v)
    gc, bc = to_c(g), to_c(beta)
    G = jnp.cumsum(gc, -1)
    diff = G[..., :, None] - G[..., None, :]
    strict = jnp.tril(jnp.ones((CHUNK, CHUNK), bool), -1)
    incl = jnp.tril(jnp.ones((CHUNK, CHUNK), bool))
    M = jnp.where(strict, bc[..., :, None] * jnp.exp(diff)
                  * jnp.einsum('nbhtd,nbhsd->nbhts', kc, kc), 0.0)
    A = jnp.where(incl, scale * jnp.exp(diff)
                  * jnp.einsum('nbhtd,nbhsd->nbhts', qc, kc), 0.0)
    I = jnp.eye(CHUNK, dtype=q.dtype)
    # (I+M)^-1 for unit lower-triangular via Newton doubling: exact after 5 iters
    # since M is strictly lower triangular (nilpotent, M^64 = 0).
    X = I - M
    for _ in range(5):
        X = X + jnp.einsum('nbhts,nbhsd->nbhtd', X, I[None, None, None] - jnp.einsum(
            'nbhts,nbhsd->nbhtd', I[None, None, None] + M, X))
    Tinv = X
    u0 = jnp.einsum('nbhts,nbhsd->nbhtd', Tinv, bc[..., None] * vc)
    w = jnp.einsum('nbhts,nbhsd->nbhtd', Tinv, (bc * jnp.exp(G))[..., None] * kc)
    kd = jnp.exp(G[..., -1:] - G)[..., None] * kc
    qd = scale * jnp.exp(G)[..., None] * qc
    gC = G[..., -1]

    S = jnp.zeros((B, H, Dk, Dv), q.dtype)
    outs = []
    egC = jnp.exp(gC)
    for n in range(N):
        u = u0[n] - jnp.einsum('bhcd,bhde->bhce', w[n], S)
        o = jnp.einsum('bhts,bhse->bhte', A[n], u) \
            + jnp.einsum('bhcd,bhde->bhce', qd[n], S)
        S = egC[n][..., None, None] * S \
            + jnp.einsum('bhcd,bhce->bhde', kd[n], u)
        outs.append(o)
    o = jnp.stack(outs, 0)
    o = jnp.moveaxis(o, 0, 2).reshape(B, H, T, Dv)
    return jnp.moveaxis(o, 1, 2)


def _shard_fn(hidden_states, W_qkvz, conv_w, g, beta, norm_weight, W_out):
    # Per-shard: HK_L key heads, HV_L value heads. g/beta precomputed host-side
    # (the tiny W_ba path) because this backend can't lower log/softplus.
    B, T, D = hidden_states.shape
    key_dim = HK_L * DK                      # 256
    qkvz = hidden_states @ W_qkvz            # [B,T,1536]
    qkvz = qkvz.reshape(B, T, HK_L, 2 * DK + 2 * RATIO * DV)
    q = qkvz[..., :DK]
    k = qkvz[..., DK:2 * DK]
    v = qkvz[..., 2 * DK:2 * DK + RATIO * DV].reshape(B, T, HV_L, DV)
    z = qkvz[..., 2 * DK + RATIO * DV:].reshape(B, T, HV_L, DV)

    mixed = jnp.concatenate([q.reshape(B, T, -1), k.reshape(B, T, -1),
                             v.reshape(B, T, -1)], -1)   # [B,T,1024]
    xp = jnp.pad(mixed, ((0, 0), (K_CONV - 1, 0), (0, 0)))
    conv = sum(conv_w[:, j] * xp[:, j:j + T, :] for j in range(K_CONV))
    mixed = jax.nn.silu(conv)
    q = mixed[..., :key_dim].reshape(B, T, HK_L, DK)
    k = mixed[..., key_dim:2 * key_dim].reshape(B, T, HK_L, DK)
    v = mixed[..., 2 * key_dim:].reshape(B, T, HV_L, DV)

    q = _l2norm(q)
    k = _l2norm(k)
    q = jnp.repeat(q, RATIO, axis=2)
    k = jnp.repeat(k, RATIO, axis=2)

    o = _chunk_gated_delta_rule(q, k, v, g, beta)

    xg = o * jax.nn.silu(z)
    xg = xg * jax.lax.rsqrt(jnp.mean(xg * xg, -1, keepdims=True) + 1e-6) * norm_weight
    return xg.reshape(B, T, -1) @ W_out


_pmapped = jax.pmap(_shard_fn)


def kernel(hidden_states, W_qkvz, W_ba, conv_w, dt_bias, A_log, norm_weight, W_out):
    hidden_states = np.asarray(hidden_states, np.float32)
    B, T, D = hidden_states.shape
    key_dim, value_dim = HK * DK, HV * DV

    # Host-side tiny path: ba = hidden @ W_ba -> g, beta  (0.25% of FLOPs; the
    # neuron backend cannot lower log/softplus in this graph).
    ba = hidden_states.reshape(-1, D).astype(np.float64) @ np.asarray(W_ba, np.float64)
    ba = ba.reshape(B, T, HK, 2 * RATIO)
    b = ba[..., :RATIO].reshape(B, T, HV)
    a = ba[..., RATIO:].reshape(B, T, HV)
    beta_full = 1.0 / (1.0 + np.exp(-b))
    sp = np.logaddexp(0.0, a + np.asarray(dt_bias, np.float64))
    g_full = (-np.exp(np.asarray(A_log, np.float64)) * sp).astype(np.float32)
    beta_full = beta_full.astype(np.float32)

    # Per-core slices.
    Wq_s = np.stack([np.asarray(W_qkvz[:, 1536 * i:1536 * (i + 1)]) for i in range(NC)])
    g_s = np.stack([g_full[..., 4 * i:4 * (i + 1)] for i in range(NC)])
    beta_s = np.stack([beta_full[..., 4 * i:4 * (i + 1)] for i in range(NC)])
    cw = np.asarray(conv_w)
    conv_s = np.stack([
        np.concatenate([cw[256 * i:256 * (i + 1)],                      # q rows
                        cw[key_dim + 256 * i:key_dim + 256 * (i + 1)],  # k rows
                        cw[2 * key_dim + 512 * i:2 * key_dim + 512 * (i + 1)]], 0)
        for i in range(NC)])
    Wout_s = np.stack([np.asarray(W_out[512 * i:512 * (i + 1)]) for i in range(NC)])
    hid_s = np.broadcast_to(hidden_states, (NC,) + hidden_states.shape)
    nw_s = np.broadcast_to(np.asarray(norm_weight), (NC, DV))

    out = _pmapped(hid_s, Wq_s, conv_s, g_s, beta_s, nw_s, Wout_s)
    return np.asarray(out, np.float32).sum(0).astype(np.float32)
